# revision 60
# baseline (speedup 1.0000x reference)
"""Multi-head attention (B=2, S=4096, D_MODEL=512, H=8) on 8 TRN2 NeuronCores.

Sharding (data + head/tensor parallel, per the problem's sharding hint):
each core owns (batch b = core//4, head-pair hp = core%4).
 - Q/K/V are batch-sharded (cores sharing b get the same activations,
   pre-transposed to [D, S] on the host so the PE can contract over D).
 - W_q/W_k/W_v are column-sharded per head pair (128 output dims/core).
 - W_o is row-sharded: each core emits a partial [S, D] output (bf16) and
   the host sums the four partials per batch + b_o (the "all-reduce"
   unshard of a row-sharded matmul).
 - Keys with mask==0 contribute nothing, so only live keys are gathered
   (padded to a multiple of 1024; pads carry mask 0 and cancel on-device).

Device pipeline per core (key performance structure):
  * Scores via ROW-TILED K=64 matmul pairs: the two heads' 64-dim q/k
    vectors live in partition halves.  Key-tiles are split into an A group
    (first nk/2 keys, projected with the natural weight layout -> head h
    dims at partitions 64h..64h+63) and a B group (second nk/2 keys,
    projected with a row-swapped weight copy -> head dims in the OTHER
    partition half).  Each score round issues one A-side MM (rows 0-63)
    and one B-side MM (rows 64-127); disjoint row-groups run concurrently
    on the PE, so MM1 streams ~256 score elements/cycle.
  * exp on ScalarE straight from 2 PSUM banks per call ([128,1024] bf16
    out, softmax scale 0.125 and a -2.0 shift folded into the activation);
    round 3 of each step computes exp on the DVE instead via a Schraudolph
    bit-trick (bits = s*A + C viewed as bf16, ~1.8% rms noise on 1/8 of
    the tiles) to relieve the ACT engine.
  * MM2 (p @ v, v column 64 = key mask yielding the softmax denominator
    as accumulator row 64) is software-pipelined one FULL step behind the
    exp stream: the PE queue is strict in-order, so an MM2 waiting on V or
    the accumulator would block later MM1s and starve exp.
  * Normalization: denominator broadcast across partitions with a rank-1
    f32r matmul + fast reciprocal; both heads' normalized rows land in one
    [128, S] stack (head0 rows 0-63, head1 rows 64-127).
  * Output projection: ONE matmul per 128-row s-tile (contraction over the
    full 128-dim stack against [W_o h0 | W_o h1]), bf16 partial out.
  * All bulk inputs are host-restaged into the exact SBUF layout so the
    few staging DMAs stream at line rate, ordered so the first exp fires
    ~17us in; the scalar DMA queue carries only 2 pre-attention DMAs
    (its ring-guard waits would block the exp stream in the ACT FIFO).
"""

import sys

for _p in ("/opt/trn_rl_repo", "/opt/pypackages"):
    if _p not in sys.path:
        sys.path.append(_p)

import numpy as np
import ml_dtypes

B = 2
S = 4096
D = 512
H = 8
DK = 64
N_CORES = 8

P = 128          # partitions
QC = 512         # q-chunk width
N_SC = S // QC   # 8 q-chunks
N_DT = D // P    # 4 D-tiles (contraction tiles for projections)

_COMPILED = {}
_LAST_IN_MAPS = None
_LAST_RESULTS = None
_LAST_NKT = None

COMPACT = True
# fp8 DoubleRow MM2 measured 3.9% output error (fp8's ~2% per-element
# noise on p and v passes straight through the softmax) vs the 2e-2
# gate -> bf16 per-tile MM2.
USE_DR = False


def _slot0(g, nkt):
    """v/mask slot for global key-tile g, head0 ordering."""
    half = nkt // 2
    return 2 * g if g < half else 2 * (g - half) + 1


def _slot1(g, nkt):
    half = nkt // 2
    return 2 * g + 1 if g < half else 2 * (g - half)


def _build(nkt: int):
    """Build + compile the per-core bass program. nkt = number of 128-wide
    key tiles (multiple of 8; 16 = 2048 live keys)."""
    import concourse.bass as bass  # noqa: F401
    from concourse.masks import make_identity
    import concourse.mybir as mybir
    import concourse.tile as tile
    from concourse import bacc

    f32 = mybir.dt.float32
    f32r = mybir.dt.float32r
    bf16 = mybir.dt.bfloat16
    fp8 = mybir.dt.float8e4
    EXP = mybir.ActivationFunctionType.Exp
    DR = mybir.MatmulPerfMode.DoubleRow

    p_dt = fp8 if USE_DR else bf16

    nk = nkt * P
    n_kc = nk // QC          # 512-key chunks (even; half A, half B)
    n_hc = n_kc // 2         # chunks per A/B group
    half_k = nk // 2
    n_rounds = nkt // 2      # k-rounds of 2 tiles

    nc = bacc.Bacc("TRN2", target_bir_lowering=False, debug=False,
                   enable_asserts=False)

    # All bulk inputs are pre-staged on the host into the exact SBUF
    # layout, so every staging DMA has multi-KB contiguous runs per
    # partition and streams at line rate (per-dt [128, cols] slices of a
    # [D, n] tensor only have 1-2KB runs -> descriptor-dominated, ~6us
    # per 256KB, which pushed the first exp out to ~47us).
    # [P, chunk, N_DT*QC]: chunk c, row p holds X.T[dt*128+p, c*512+col]
    QS = nc.dram_tensor("QS", [P, N_SC, N_DT * QC], bf16,
                        kind="ExternalInput").ap()
    KS = nc.dram_tensor("KS", [P, n_kc, N_DT * QC], bf16,
                        kind="ExternalInput").ap()
    VS = nc.dram_tensor("VS", [P, n_kc, N_DT * QC], bf16,
                        kind="ExternalInput").ap()
    # weights: slots [wkn, wks, wqn, wqs, wv] as [P, dt*128+c], slot 5 =
    # merged W_o stack [128, 512]
    WALL = nc.dram_tensor("WALL", [P, 6, D], bf16,
                          kind="ExternalInput").ap()
    # cols 0-4 = biases (bq, bqs, bk, bks, bv), then mask0, mask1
    SMALL = nc.dram_tensor("SMALL", [P, 5 + 2 * nkt], f32,
                           kind="ExternalInput").ap()
    OUT = nc.dram_tensor("OUT", [S, D], bf16, kind="ExternalOutput").ap()

    with tile.TileContext(nc) as tc:
        with tc.tile_pool(name="persist", bufs=1) as persist:
            # ---- persistent SBUF tensors ----
            # per-head q [128, S]: head h dims duplicated in BOTH partition
            # halves (A-side MMs read 0:64, B-side read 64:128)
            qh_sb = [persist.tile([P, S], bf16, name=f"qh{h}_sb")
                     for h in range(2)]
            # per-head k [128, nk/2]: partitions 0-63 = one key group's
            # head-h dims, 64-127 = the other group's (A/B per head)
            kh_sb = [persist.tile([P, half_k], bf16, name=f"kh{h}_sb")
                     for h in range(2)]
            # v tiles in slot order, padded inner dim 80 (DoubleRow needs
            # 16B-aligned pair stride); col 64 = key mask
            v0_sb = persist.tile([P, nkt, 80], p_dt)
            v1_sb = persist.tile([P, nkt, 80], p_dt)
            # normalized attention stack [h0 rows 0-63 | h1 rows 64-127]
            a_sb = persist.tile([P, S], bf16)
            # raw input staging (SBUF layout == DRAM layout, see above)
            kin_sb = persist.tile([P, n_kc, N_DT * QC], bf16)
            vin_sb = persist.tile([P, n_kc, N_DT * QC], bf16)
            qin_sb = persist.tile([P, N_SC, N_DT * QC], bf16)
            wall_sb = persist.tile([P, 6, D], bf16)
            small_sb = persist.tile([P, 5 + 2 * nkt], f32)
            ident_sb = persist.tile([P, P], bf16)
            ones64_sb = persist.tile([65, DK], f32r)
            ones64_f = persist.tile([65, DK], f32)
            dummy_sb = persist.tile([1, 16], f32)
            eshift_sb = persist.tile([P, 1], f32)   # exp bias (-2.0)
            nc.vector.memset(eshift_sb, -2.0)
            wkn_sb, wks_sb = wall_sb[:, 0, :], wall_sb[:, 1, :]
            wqn_sb, wqs_sb = wall_sb[:, 2, :], wall_sb[:, 3, :]
            wv_sb = wall_sb[:, 4, :]
            wos_sb = wall_sb[:, 5, :]
            bq_sb, bqs_sb = small_sb[:, 0:1], small_sb[:, 1:2]
            bk_sb, bks_sb = small_sb[:, 2:3], small_sb[:, 3:4]
            bv_sb = small_sb[:, 4:5]
            mask0_sb = small_sb[:, 5:5 + nkt]
            mask1_sb = small_sb[:, 5 + nkt:5 + 2 * nkt]

            # warm the ACT exp table during staging
            nc.vector.memset(dummy_sb, 0.0)
            nc.scalar.activation(dummy_sb, dummy_sb, EXP, bias=0.0,
                                 scale=1.0)
            make_identity(nc, ident_sb)
            nc.vector.memset(ones64_f, 1.0)
            nc.vector.tensor_copy(out=ones64_sb, in_=ones64_f)

            # ---------------- staging DMAs ----------------
            # Spread the critical path (weights -> K -> Q chunk 0) over
            # three queues, finest pieces first.  The scalar queue
            # carries only TWO pre-attention DMAs (its ring-guard waits
            # would otherwise block the exp stream in the ACT FIFO).
            # sync: W(kn,ks,qn,qs,v), K-A chunk 0, K-A rest, V-A0, V-A
            # rest, wos
            nc.sync.dma_start(out=wall_sb[:, 0:5, :], in_=WALL[:, 0:5, :])
            nc.sync.dma_start(out=kin_sb[:, 0:1, :], in_=KS[:, 0:1, :])
            if n_hc > 1:
                nc.sync.dma_start(out=kin_sb[:, 1:n_hc, :],
                                  in_=KS[:, 1:n_hc, :])
            nc.sync.dma_start(out=vin_sb[:, 0:1, :], in_=VS[:, 0:1, :])
            if n_hc > 1:
                nc.sync.dma_start(out=vin_sb[:, 1:n_hc, :],
                                  in_=VS[:, 1:n_hc, :])
            nc.sync.dma_start(out=wall_sb[:, 5, :], in_=WALL[:, 5, :])
            # gpsimd (slow SWDGE): smalls, V-B0, V-B rest, Q 5-7
            nc.gpsimd.dma_start(out=small_sb, in_=SMALL)
            nc.gpsimd.dma_start(out=vin_sb[:, n_hc:n_hc + 1, :],
                                in_=VS[:, n_hc:n_hc + 1, :])
            if n_hc > 1:
                nc.gpsimd.dma_start(out=vin_sb[:, n_hc + 1:n_kc, :],
                                    in_=VS[:, n_hc + 1:n_kc, :])
            nc.gpsimd.dma_start(out=qin_sb[:, 5:N_SC, :],
                                in_=QS[:, 5:N_SC, :])
            # scalar (HWDGE, fast): K-B chunk 0 (gates MM1 round 0), Q
            # chunk 0, K-B rest, Q chunk 1 -- all four ring-guard waits
            # resolve before the first exp
            nc.scalar.dma_start(out=kin_sb[:, n_hc:n_hc + 1, :],
                                in_=KS[:, n_hc:n_hc + 1, :])
            nc.scalar.dma_start(out=qin_sb[:, 0:1, :], in_=QS[:, 0:1, :])
            if n_hc > 1:
                nc.scalar.dma_start(out=kin_sb[:, n_hc + 1:n_kc, :],
                                    in_=KS[:, n_hc + 1:n_kc, :])
            nc.scalar.dma_start(out=qin_sb[:, 1:2, :], in_=QS[:, 1:2, :])

            # v mask columns (slot order, per head)
            nc.vector.tensor_copy(out=v0_sb[:, :, 64], in_=mask0_sb)
            nc.vector.tensor_copy(out=v1_sb[:, :, 64], in_=mask1_sb)

            # ================= projections =================
            pin = tc.alloc_tile_pool(name="pin", bufs=2)
            tx = tc.alloc_tile_pool(name="tx", bufs=2, space="PSUM")

            def kproj(c):
                """Project K chunk c (keys c*512..) into kh_0/kh_1."""
                swapped = c >= n_hc
                w = wks_sb if swapped else wkn_sb
                b = bks_sb if swapped else bk_sb
                osl = slice((c - n_hc) * QC, (c - n_hc + 1) * QC) \
                    if swapped else slice(c * QC, (c + 1) * QC)
                ps = tx.tile([P, QC], f32, tag="tx", name="kps", bufs=2)
                for dt in range(N_DT):
                    nc.tensor.matmul(ps, lhsT=w[:, dt * P:(dt + 1) * P],
                                     rhs=kin_sb[:, c, dt * QC:(dt + 1) * QC],
                                     start=(dt == 0), stop=(dt == N_DT - 1))
                if swapped:
                    # rows 0-63 = head1 dims, 64-127 = head0 dims
                    nc.vector.tensor_scalar_add(
                        out=kh_sb[1][0:DK, osl], in0=ps[0:DK, :],
                        scalar1=b[0:DK, :])
                    nc.vector.tensor_scalar_add(
                        out=kh_sb[0][DK:P, osl], in0=ps[DK:P, :],
                        scalar1=b[DK:P, :])
                else:
                    nc.vector.tensor_scalar_add(
                        out=kh_sb[0][0:DK, osl], in0=ps[0:DK, :],
                        scalar1=b[0:DK, :])
                    nc.vector.tensor_scalar_add(
                        out=kh_sb[1][DK:P, osl], in0=ps[DK:P, :],
                        scalar1=b[DK:P, :])

            def qproj(sc, swapped):
                """Project Q chunk sc with normal or swapped weights."""
                w = wqs_sb if swapped else wqn_sb
                b = bqs_sb if swapped else bq_sb
                ssl = slice(sc * QC, (sc + 1) * QC)
                ps = tx.tile([P, QC], f32, tag="tx", name="qps", bufs=2)
                for dt in range(N_DT):
                    nc.tensor.matmul(ps, lhsT=w[:, dt * P:(dt + 1) * P],
                                     rhs=qin_sb[:, sc, dt * QC:(dt + 1) * QC],
                                     start=(dt == 0), stop=(dt == N_DT - 1))
                if swapped:
                    nc.vector.tensor_scalar_add(
                        out=qh_sb[1][0:DK, ssl], in0=ps[0:DK, :],
                        scalar1=b[0:DK, :])
                    nc.vector.tensor_scalar_add(
                        out=qh_sb[0][DK:P, ssl], in0=ps[DK:P, :],
                        scalar1=b[DK:P, :])
                else:
                    nc.vector.tensor_scalar_add(
                        out=qh_sb[0][0:DK, ssl], in0=ps[0:DK, :],
                        scalar1=b[0:DK, :])
                    nc.vector.tensor_scalar_add(
                        out=qh_sb[1][DK:P, ssl], in0=ps[DK:P, :],
                        scalar1=b[DK:P, :])

            def vproj(vc):
                """Project V chunk vc, transpose per 128-key block into the
                per-head slot-ordered v tiles (masked, fp8)."""
                ps = tx.tile([P, QC], f32, tag="tx", name="vps", bufs=2)
                for dt in range(N_DT):
                    nc.tensor.matmul(ps, lhsT=wv_sb[:, dt * P:(dt + 1) * P],
                                     rhs=vin_sb[:, vc, dt * QC:(dt + 1) * QC],
                                     start=(dt == 0), stop=(dt == N_DT - 1))
                vt_sb = pin.tile([P, QC], bf16, tag="vt", bufs=2,
                                 name="vt_sb")
                nc.vector.tensor_scalar_add(out=vt_sb, in0=ps,
                                            scalar1=bv_sb)
                for st4 in range(QC // P):
                    g = vc * (QC // P) + st4
                    s0, s1 = _slot0(g, nkt), _slot1(g, nkt)
                    tp = tx.tile([P, P], bf16, tag="tx", name="tp", bufs=2)
                    nc.tensor.transpose(
                        tp, vt_sb[:, st4 * P:(st4 + 1) * P], ident_sb)
                    nc.vector.tensor_scalar_mul(
                        out=v0_sb[:, s0, 0:DK], in0=tp[:, 0:DK],
                        scalar1=mask0_sb[:, s0:s0 + 1])
                    nc.vector.tensor_scalar_mul(
                        out=v1_sb[:, s1, 0:DK], in0=tp[:, DK:P],
                        scalar1=mask1_sb[:, s1:s1 + 1])

            # initial projections in DMA-arrival order: B0 (gpsimd lands
            # first), Q chunk 0, A0, B-rest, A-rest.  V projections are
            # deferred into the attention loop (MM2s are software-
            # pipelined one step behind, so V has a full step of slack).
            kproj(n_hc)
            qproj(0, False)
            qproj(0, True)
            kproj(0)

            # ================= attention =================
            # exp runs r in the current step while MM2/norm run one FULL
            # step behind (the PE queue is strict in-order, so an MM2
            # waiting on V or acc would block later MM1s and starve the
            # exp stream; one step of lag gives V/norm ~9us of slack).
            # Rounds 3 and 7 of each step compute exp on the DVE via a
            # Schraudolph bit-trick (bits = s*A + C viewed as bf16,
            # ~1.8% rms noise) to relieve the ACT engine.
            EXPA = float(128 * 0.125 * 1.4426950408889634)
            EXPC = float(16256 - 128 * 2 * 1.4426950408889634 - 7.41)
            DVE_ROUNDS = {3}
            u16 = mybir.dt.uint16
            with tc.tile_pool(name="aps", bufs=1, space="PSUM") as aps, \
                 tc.tile_pool(name="asb", bufs=1) as asb:
                v_sbs = (v0_sb, v1_sb)
                steps = [(h, sc) for h in (0, 1) for sc in range(N_SC)]
                NS = len(steps)

                def emit_mm1(h, sc, r):
                    """Row-tiled score pair for round r (slots 2r, 2r+1)."""
                    ssl = slice(sc * QC, (sc + 1) * QC)
                    s_ps = aps.tile([P, 2, QC], f32, tag="mm1", bufs=2,
                                    name="s_ps")
                    for j in (2 * r, 2 * r + 1):
                        p = j % 2
                        i = j // 2
                        psl = slice(DK * p, DK * (p + 1))
                        nc.tensor.matmul(
                            s_ps[:, p, :],
                            lhsT=kh_sb[h][psl, i * P:(i + 1) * P],
                            rhs=qh_sb[h][psl, ssl],
                            start=True, stop=True)
                    return s_ps

                def emit_outproj_tile(sc, st4):
                    st = sc * (QC // P) + st4
                    tsl = slice(st * P, (st + 1) * P)
                    po = tx.tile([P, D], f32, tag="tx", bufs=2,
                                 name="po")
                    nc.tensor.matmul(po, lhsT=a_sb[:, tsl],
                                     rhs=wos_sb, start=True, stop=True)
                    osb = asb.tile([P, D], bf16, tag="osb", bufs=4)
                    nc.any.tensor_copy(out=osb, in_=po)
                    eng = nc.gpsimd if st4 == 3 else nc.sync
                    eng.dma_start(out=OUT[tsl, :], in_=osb)

                p_store = {}
                s_ps_next = emit_mm1(0, 0, 0)
                acc = None
                pending = []
                for si in range(NS + 1):
                    cur = steps[si] if si < NS else None
                    prev = steps[si - 1] if si > 0 else None
                    for r in range(n_rounds):
                        if cur is not None:
                            h, sc = cur
                            s_ps = s_ps_next
                            p_sb = asb.tile([P, 2, QC], p_dt, tag="p",
                                            bufs=12)
                            p_store[(si, r)] = p_sb
                            if r in DVE_ROUNDS:
                                nc.vector.tensor_scalar(
                                    p_sb[:, :, :].bitcast(u16), s_ps,
                                    EXPA, EXPC,
                                    mybir.AluOpType.mult,
                                    mybir.AluOpType.add)
                            else:
                                nc.scalar.activation(
                                    p_sb, s_ps, EXP, bias=eshift_sb[:, :],
                                    scale=0.125)
                            if not (si == NS - 1 and r == n_rounds - 1):
                                nh, nsc = steps[si + 1] \
                                    if r == n_rounds - 1 else cur
                                nr = 0 if r == n_rounds - 1 else r + 1
                                s_ps_next = emit_mm1(nh, nsc, nr)
                            # late K projections: emitted before the
                            # MM1(r4) emission at r==3 (emission order
                            # defines dependency order)
                            if si == 0 and n_hc > 1 and r == 2:
                                for i_ in range(1, n_hc):
                                    kproj(i_)
                                    kproj(n_hc + i_)

                        if prev is not None:
                            ph, psc = prev
                            # late V projections (DMA-arrival paced)
                            if si == 1:
                                if r == 0:
                                    vproj(0)
                                elif r == 3 and n_hc > 1:
                                    vproj(1)
                            if r == 0:
                                acc = aps.tile([65, QC], f32, tag="acc",
                                               bufs=2, name="acc")
                            pv = v_sbs[ph]
                            p_prev = p_store.pop((si - 1, r))
                            for j in (0, 1):
                                nc.tensor.matmul(
                                    acc[0:65, :],
                                    lhsT=pv[:, 2 * r + j, 0:65],
                                    rhs=p_prev[:, j, :],
                                    start=(r == 0 and j == 0),
                                    stop=(r == n_rounds - 1 and j == 1))
                            if r == n_rounds - 1:
                                # normalize prev step: attn = acc[0:64] /
                                # acc[64] via rank-1 f32r broadcast + fast
                                # reciprocal
                                pssl = slice(psc * QC, (psc + 1) * QC)
                                den = asb.tile([65, QC], f32r, tag="den",
                                               bufs=2)
                                nc.vector.tensor_copy(out=den[64:65, :],
                                                      in_=acc[64:65, :])
                                rb_ps = tx.tile([P, QC], f32, tag="tx",
                                                bufs=2, name="rb_ps")
                                nc.tensor.matmul(
                                    rb_ps[0:64, :],
                                    lhsT=ones64_sb[64:65, :],
                                    rhs=den[64:65, :],
                                    start=True, stop=True)
                                recb = asb.tile([64, QC], f32, tag="recb",
                                                bufs=2)
                                nc.vector.reciprocal_approx_fast(
                                    out=recb, in_=rb_ps[0:64, :])
                                nc.any.tensor_mul(
                                    out=a_sb[DK * ph:DK * (ph + 1), pssl],
                                    in0=acc[0:64, :], in1=recb)
                                if ph == 1:
                                    pending.append((psc, si))
                        if cur is not None:
                            h, sc = cur
                            # B-group V projections late in step 0
                            if si == 0:
                                if r == n_rounds - 2:
                                    vproj(n_hc)
                                elif r == n_rounds - 1 and n_hc > 1:
                                    vproj(n_hc + 1)
                            # mid-stream Q staging on the scalar queue
                            if si == 0 and r == 4 and N_SC > 2:
                                nc.scalar.dma_start(out=qin_sb[:, 2:3, :],
                                                    in_=QS[:, 2:3, :])
                            elif si == 0 and r == 6 and N_SC > 3:
                                nc.scalar.dma_start(out=qin_sb[:, 3:4, :],
                                                    in_=QS[:, 3:4, :])
                            elif si == 1 and r == 1 and N_SC > 4:
                                nc.scalar.dma_start(out=qin_sb[:, 4:5, :],
                                                    in_=QS[:, 4:5, :])
                            # next q chunk during h0: single projection +
                            # partition-dup via scalar-queue SBUF DMAs
                            if h == 0 and sc + 1 < N_SC and \
                                    r == (n_rounds - 2 if sc == 0 else 2):
                                qproj(sc + 1, False)
                                nssl = slice((sc + 1) * QC, (sc + 2) * QC)
                                nc.scalar.dma_start(
                                    out=qh_sb[0][DK:P, nssl],
                                    in_=qh_sb[0][0:DK, nssl])
                                nc.scalar.dma_start(
                                    out=qh_sb[1][0:DK, nssl],
                                    in_=qh_sb[1][DK:P, nssl])
                    while pending and si - pending[0][1] >= 1:
                        for st4 in range(QC // P):
                            emit_outproj_tile(pending[0][0], st4)
                        pending.pop(0)
                for psc_o, _ in pending:
                    for st4 in range(QC // P):
                        emit_outproj_tile(psc_o, st4)
            pin.release()
            tx.release()

    nc.compile()
    return nc


def _get_compiled(nkt: int):
    if nkt not in _COMPILED:
        _COMPILED[nkt] = _build(nkt)
    return _COMPILED[nkt]


def kernel(Q, K, V, mask, W_q, b_q, W_k, b_k, W_v, b_v, W_o, b_o):
    from concourse import bass_utils

    bf16 = ml_dtypes.bfloat16
    mask = np.asarray(mask)
    if COMPACT:
        # keys with mask==0 contribute nothing: gather live keys, padded
        # to a multiple of 1024 (A/B halves of 512-key chunks)
        idxs = [np.flatnonzero(mask[b]) for b in range(B)]
        nkt = max(1, -(-max(len(ix) for ix in idxs) // P))
        nkt = min(-(-nkt // 8) * 8, S // P)
    else:
        idxs = None
        nkt = S // P
    nk = nkt * P

    Q = np.asarray(Q, np.float32)
    K = np.asarray(K, np.float32)
    V = np.asarray(V, np.float32)
    W_q = np.asarray(W_q, np.float32)
    W_k = np.asarray(W_k, np.float32)
    W_v = np.asarray(W_v, np.float32)
    W_o = np.asarray(W_o, np.float32)
    b_q = np.asarray(b_q, np.float32)
    b_k = np.asarray(b_k, np.float32)
    b_v = np.asarray(b_v, np.float32)
    b_o = np.asarray(b_o, np.float32)

    nc = _get_compiled(nkt)

    # slot permutation for the per-head mask tiles
    halft = nkt // 2
    slot0 = [0] * nkt
    for g in range(nkt):
        slot0[g] = 2 * g if g < halft else 2 * (g - halft) + 1
    slot1 = [0] * nkt
    for g in range(nkt):
        slot1[g] = 2 * g + 1 if g < halft else 2 * (g - halft)

    def stage_x(xt, nchunks):
        # [D, n] -> [P, chunk, N_DT*QC] with [p, c, dt*QC+col] =
        # xt[dt*P+p, c*QC+col]
        return np.ascontiguousarray(
            xt.reshape(N_DT, P, nchunks, QC).transpose(1, 2, 0, 3)
            .reshape(P, nchunks, N_DT * QC)).astype(bf16)

    def stage_w(w):
        # W slice [128out, 512in] -> [P, dt*P+c] = W.T[dt*P+p, c]
        return w.T.reshape(N_DT, P, P).transpose(1, 0, 2).reshape(P, D)

    n_kc = nk // QC
    in_maps = []
    for core in range(N_CORES):
        b = core // 4
        hp = core % 4
        rsl = slice(hp * P, (hp + 1) * P)   # rows of W_q/W_k/W_v, 2 heads
        h0, h1 = 2 * hp, 2 * hp + 1

        if COMPACT:
            ix = idxs[b]
            nkeep = len(ix)
            ixp = np.zeros(nk, np.int64)
            ixp[:nkeep] = ix[:nk]
            kt = K[b][ixp, :].T
            vt = V[b][ixp, :].T
            mvec = (np.arange(nk) < min(nkeep, nk)).astype(np.float32)
        else:
            kt = K[b].T
            vt = V[b].T
            mvec = mask[b].astype(np.float32)

        wq = W_q[rsl, :]                     # [128, 512], rows = head dims
        wk = W_k[rsl, :]
        swap = np.concatenate([np.arange(DK, P), np.arange(0, DK)])
        mt = mvec.reshape(nkt, P).T          # [128, nkt] global tile order
        # permute columns: MASKh[:, slot] = mask of global tile g
        m0 = np.zeros((P, nkt), np.float32)
        m1 = np.zeros((P, nkt), np.float32)
        for g in range(nkt):
            m0[:, slot0[g]] = mt[:, g]
            m1[:, slot1[g]] = mt[:, g]

        wos = np.zeros((P, D), np.float32)
        wos[0:DK, :] = W_o[:, h0 * DK:(h0 + 1) * DK].T
        wos[DK:P, :] = W_o[:, h1 * DK:(h1 + 1) * DK].T

        wall = np.stack([
            stage_w(wk), stage_w(wk[swap, :]),
            stage_w(wq), stage_w(wq[swap, :]),
            stage_w(W_v[rsl, :]), wos,
        ], axis=1)                            # [P, 6, D]

        small = np.zeros((P, 5 + 2 * nkt), np.float32)
        small[:, 0] = b_q[rsl]
        small[:, 1] = b_q[rsl][swap]
        small[:, 2] = b_k[rsl]
        small[:, 3] = b_k[rsl][swap]
        small[:, 4] = b_v[rsl]
        small[:, 5:5 + nkt] = m0
        small[:, 5 + nkt:] = m1

        in_maps.append({
            "QS": stage_x(Q[b].T, N_SC),
            "KS": stage_x(kt, n_kc),
            "VS": stage_x(vt, n_kc),
            "WALL": np.ascontiguousarray(wall).astype(bf16),
            "SMALL": small,
        })

    global _LAST_IN_MAPS, _LAST_RESULTS, _LAST_NKT
    _LAST_IN_MAPS = in_maps
    _LAST_NKT = nkt

    res = bass_utils.run_bass_kernel_spmd(
        nc, in_maps, core_ids=list(range(N_CORES)))

    _LAST_RESULTS = res.results

    out = np.zeros((B, S, D), np.float32)
    for core in range(N_CORES):
        out[core // 4] += np.asarray(res.results[core]["OUT"], np.float32)
    out += b_o[None, None, :]
    return out


# revision 61
# speedup vs baseline: 1.1462x; 1.1462x over previous
"""Multi-head attention (B=2, S=4096, D_MODEL=512, H=8) on 8 TRN2 NeuronCores.

Sharding (data + head/tensor parallel, per the problem's sharding hint):
each core owns (batch b = core//4, head-pair hp = core%4).
 - Q/K/V are batch-sharded (cores sharing b get the same activations,
   pre-transposed to [D, S] on the host so the PE can contract over D).
 - W_q/W_k/W_v are column-sharded per head pair (128 output dims/core).
 - W_o is row-sharded: each core emits a partial [S, D] output (bf16) and
   the host sums the four partials per batch + b_o (the "all-reduce"
   unshard of a row-sharded matmul).
 - Keys with mask==0 contribute nothing, so only live keys are gathered
   (padded to a multiple of 1024; pads carry mask 0 and cancel on-device).

Device pipeline per core (key performance structure):
  * Scores via ROW-TILED K=64 matmul pairs: the two heads' 64-dim q/k
    vectors live in partition halves.  Key-tiles are split into an A group
    (first nk/2 keys, projected with the natural weight layout -> head h
    dims at partitions 64h..64h+63) and a B group (second nk/2 keys,
    projected with a row-swapped weight copy -> head dims in the OTHER
    partition half).  Each score round issues one A-side MM (rows 0-63)
    and one B-side MM (rows 64-127); disjoint row-groups run concurrently
    on the PE, so MM1 streams ~256 score elements/cycle.
  * exp on ScalarE straight from 2 PSUM banks per call ([128,1024] bf16
    out, softmax scale 0.125 and a -2.0 shift folded into the activation);
    round 3 of each step computes exp on the DVE instead via a Schraudolph
    bit-trick (bits = s*A + C viewed as bf16, ~1.8% rms noise on 1/8 of
    the tiles) to relieve the ACT engine.
  * MM2 (p @ v, v column 64 = key mask yielding the softmax denominator
    as accumulator row 64) is software-pipelined one FULL step behind the
    exp stream: the PE queue is strict in-order, so an MM2 waiting on V or
    the accumulator would block later MM1s and starve exp.
  * Normalization: denominator broadcast across partitions with a rank-1
    f32r matmul + fast reciprocal; both heads' normalized rows land in one
    [128, S] stack (head0 rows 0-63, head1 rows 64-127).
  * Output projection: ONE matmul per 128-row s-tile (contraction over the
    full 128-dim stack against [W_o h0 | W_o h1]), bf16 partial out.
  * All bulk inputs are host-restaged into the exact SBUF layout so the
    few staging DMAs stream at line rate, ordered so the first exp fires
    ~17us in; the scalar DMA queue carries only 2 pre-attention DMAs
    (its ring-guard waits would block the exp stream in the ACT FIFO).
"""

import sys

for _p in ("/opt/trn_rl_repo", "/opt/pypackages"):
    if _p not in sys.path:
        sys.path.append(_p)

import numpy as np
import ml_dtypes

B = 2
S = 4096
D = 512
H = 8
DK = 64
N_CORES = 8

P = 128          # partitions
QC = 512         # q-chunk width
N_SC = S // QC   # 8 q-chunks
N_DT = D // P    # 4 D-tiles (contraction tiles for projections)

_COMPILED = {}
_LAST_IN_MAPS = None
_LAST_RESULTS = None
_LAST_NKT = None

COMPACT = True
# fp8 DoubleRow MM2 measured 3.9% output error (fp8's ~2% per-element
# noise on p and v passes straight through the softmax) vs the 2e-2
# gate -> bf16 per-tile MM2.
USE_DR = False


def _slot0(g, nkt):
    """v/mask slot for global key-tile g, head0 ordering."""
    half = nkt // 2
    return 2 * g if g < half else 2 * (g - half) + 1


def _slot1(g, nkt):
    half = nkt // 2
    return 2 * g + 1 if g < half else 2 * (g - half)


def _build(nkt: int):
    """Build + compile the per-core bass program. nkt = number of 128-wide
    key tiles (multiple of 8; 16 = 2048 live keys)."""
    import concourse.bass as bass  # noqa: F401
    from concourse.masks import make_identity
    import concourse.mybir as mybir
    import concourse.tile as tile
    from concourse import bacc

    f32 = mybir.dt.float32
    f32r = mybir.dt.float32r
    bf16 = mybir.dt.bfloat16
    fp8 = mybir.dt.float8e4
    EXP = mybir.ActivationFunctionType.Exp
    DR = mybir.MatmulPerfMode.DoubleRow

    p_dt = fp8 if USE_DR else bf16

    nk = nkt * P
    n_kc = nk // QC          # 512-key chunks (even; half A, half B)
    n_hc = n_kc // 2         # chunks per A/B group
    half_k = nk // 2
    n_rounds = nkt // 2      # k-rounds of 2 tiles

    nc = bacc.Bacc("TRN2", target_bir_lowering=False, debug=False,
                   enable_asserts=False)

    # All bulk inputs are pre-staged on the host into the exact SBUF
    # layout, so every staging DMA has multi-KB contiguous runs per
    # partition and streams at line rate (per-dt [128, cols] slices of a
    # [D, n] tensor only have 1-2KB runs -> descriptor-dominated, ~6us
    # per 256KB, which pushed the first exp out to ~47us).
    # [P, chunk, N_DT*QC]: chunk c, row p holds X.T[dt*128+p, c*512+col]
    QS = nc.dram_tensor("QS", [P, N_SC, N_DT * QC], bf16,
                        kind="ExternalInput").ap()
    KS = nc.dram_tensor("KS", [P, n_kc, N_DT * QC], bf16,
                        kind="ExternalInput").ap()
    VS = nc.dram_tensor("VS", [P, n_kc, N_DT * QC], bf16,
                        kind="ExternalInput").ap()
    # weights: slots [wkn, wks, wqn, wqs, wv] as [P, dt*128+c], slot 5 =
    # merged W_o stack [128, 512]
    WALL = nc.dram_tensor("WALL", [P, 6, D], bf16,
                          kind="ExternalInput").ap()
    # cols 0-4 = biases (bq, bqs, bk, bks, bv), then mask0, mask1
    SMALL = nc.dram_tensor("SMALL", [P, 5 + 2 * nkt], f32,
                           kind="ExternalInput").ap()
    OUT = nc.dram_tensor("OUT", [S, D], bf16, kind="ExternalOutput").ap()

    with tile.TileContext(nc) as tc:
        with tc.tile_pool(name="persist", bufs=1) as persist:
            # ---- persistent SBUF tensors ----
            # per-head q [128, S]: head h dims duplicated in BOTH partition
            # halves (A-side MMs read 0:64, B-side read 64:128)
            qh_sb = [persist.tile([P, S], bf16, name=f"qh{h}_sb")
                     for h in range(2)]
            # per-head k [128, nk/2]: partitions 0-63 = one key group's
            # head-h dims, 64-127 = the other group's (A/B per head)
            kh_sb = [persist.tile([P, half_k], bf16, name=f"kh{h}_sb")
                     for h in range(2)]
            # v tiles in slot order, padded inner dim 80 (DoubleRow needs
            # 16B-aligned pair stride); col 64 = key mask
            v0_sb = persist.tile([P, nkt, 80], p_dt)
            v1_sb = persist.tile([P, nkt, 80], p_dt)
            # normalized attention stack [h0 rows 0-63 | h1 rows 64-127]
            a_sb = persist.tile([P, S], bf16)
            # raw input staging (SBUF layout == DRAM layout, see above)
            kin_sb = persist.tile([P, n_kc, N_DT * QC], bf16)
            vin_sb = persist.tile([P, n_kc, N_DT * QC], bf16)
            qin_sb = persist.tile([P, N_SC, N_DT * QC], bf16)
            wall_sb = persist.tile([P, 6, D], bf16)
            small_sb = persist.tile([P, 5 + 2 * nkt], f32)
            ident_sb = persist.tile([P, P], bf16)
            ones64_sb = persist.tile([65, DK], f32r)
            ones64_f = persist.tile([65, DK], f32)
            dummy_sb = persist.tile([1, 16], f32)
            eshift_sb = persist.tile([P, 1], f32)   # exp bias (-2.0)
            nc.vector.memset(eshift_sb, -2.0)
            wkn_sb, wks_sb = wall_sb[:, 0, :], wall_sb[:, 1, :]
            wqn_sb, wqs_sb = wall_sb[:, 2, :], wall_sb[:, 3, :]
            wv_sb = wall_sb[:, 4, :]
            wos_sb = wall_sb[:, 5, :]
            bq_sb, bqs_sb = small_sb[:, 0:1], small_sb[:, 1:2]
            bk_sb, bks_sb = small_sb[:, 2:3], small_sb[:, 3:4]
            bv_sb = small_sb[:, 4:5]
            mask0_sb = small_sb[:, 5:5 + nkt]
            mask1_sb = small_sb[:, 5 + nkt:5 + 2 * nkt]

            # warm the ACT exp table during staging
            nc.vector.memset(dummy_sb, 0.0)
            nc.scalar.activation(dummy_sb, dummy_sb, EXP, bias=0.0,
                                 scale=1.0)
            make_identity(nc, ident_sb)
            nc.vector.memset(ones64_f, 1.0)
            nc.vector.tensor_copy(out=ones64_sb, in_=ones64_f)

            # ---------------- staging DMAs ----------------
            # Spread the critical path (weights -> K -> Q chunk 0) over
            # three queues, finest pieces first.  The scalar queue
            # carries only TWO pre-attention DMAs (its ring-guard waits
            # would otherwise block the exp stream in the ACT FIFO).
            # sync: W(kn,ks,qn,qs,v), K-A chunk 0, K-A rest, V-A0, V-A
            # rest, wos
            nc.sync.dma_start(out=wall_sb[:, 0:5, :], in_=WALL[:, 0:5, :])
            nc.sync.dma_start(out=kin_sb[:, 0:1, :], in_=KS[:, 0:1, :])
            if n_hc > 1:
                nc.sync.dma_start(out=kin_sb[:, 1:n_hc, :],
                                  in_=KS[:, 1:n_hc, :])
            nc.sync.dma_start(out=vin_sb[:, 0:1, :], in_=VS[:, 0:1, :])
            if n_hc > 1:
                nc.sync.dma_start(out=vin_sb[:, 1:n_hc, :],
                                  in_=VS[:, 1:n_hc, :])
            nc.sync.dma_start(out=wall_sb[:, 5, :], in_=WALL[:, 5, :])
            # gpsimd: smalls, K-B chunk 0, K-B rest, V-B0, V-B rest, Q 5-7
            nc.gpsimd.dma_start(out=small_sb, in_=SMALL)
            nc.gpsimd.dma_start(out=kin_sb[:, n_hc:n_hc + 1, :],
                                in_=KS[:, n_hc:n_hc + 1, :])
            if n_hc > 1:
                nc.gpsimd.dma_start(out=kin_sb[:, n_hc + 1:n_kc, :],
                                    in_=KS[:, n_hc + 1:n_kc, :])
            nc.gpsimd.dma_start(out=vin_sb[:, n_hc:n_hc + 1, :],
                                in_=VS[:, n_hc:n_hc + 1, :])
            if n_hc > 1:
                nc.gpsimd.dma_start(out=vin_sb[:, n_hc + 1:n_kc, :],
                                    in_=VS[:, n_hc + 1:n_kc, :])
            nc.gpsimd.dma_start(out=qin_sb[:, 5:N_SC, :],
                                in_=QS[:, 5:N_SC, :])
            # scalar: Q chunks 0-1 only
            nc.scalar.dma_start(out=qin_sb[:, 0:1, :], in_=QS[:, 0:1, :])
            nc.scalar.dma_start(out=qin_sb[:, 1:2, :], in_=QS[:, 1:2, :])

            # v mask columns (slot order, per head)
            nc.vector.tensor_copy(out=v0_sb[:, :, 64], in_=mask0_sb)
            nc.vector.tensor_copy(out=v1_sb[:, :, 64], in_=mask1_sb)

            # ================= projections =================
            pin = tc.alloc_tile_pool(name="pin", bufs=2)
            tx = tc.alloc_tile_pool(name="tx", bufs=2, space="PSUM")

            def kproj(c):
                """Project K chunk c (keys c*512..) into kh_0/kh_1."""
                swapped = c >= n_hc
                w = wks_sb if swapped else wkn_sb
                b = bks_sb if swapped else bk_sb
                osl = slice((c - n_hc) * QC, (c - n_hc + 1) * QC) \
                    if swapped else slice(c * QC, (c + 1) * QC)
                ps = tx.tile([P, QC], f32, tag="tx", name="kps", bufs=2)
                for dt in range(N_DT):
                    nc.tensor.matmul(ps, lhsT=w[:, dt * P:(dt + 1) * P],
                                     rhs=kin_sb[:, c, dt * QC:(dt + 1) * QC],
                                     start=(dt == 0), stop=(dt == N_DT - 1))
                if swapped:
                    # rows 0-63 = head1 dims, 64-127 = head0 dims
                    nc.vector.tensor_scalar_add(
                        out=kh_sb[1][0:DK, osl], in0=ps[0:DK, :],
                        scalar1=b[0:DK, :])
                    nc.vector.tensor_scalar_add(
                        out=kh_sb[0][DK:P, osl], in0=ps[DK:P, :],
                        scalar1=b[DK:P, :])
                else:
                    nc.vector.tensor_scalar_add(
                        out=kh_sb[0][0:DK, osl], in0=ps[0:DK, :],
                        scalar1=b[0:DK, :])
                    nc.vector.tensor_scalar_add(
                        out=kh_sb[1][DK:P, osl], in0=ps[DK:P, :],
                        scalar1=b[DK:P, :])

            def qproj(sc, swapped):
                """Project Q chunk sc with normal or swapped weights."""
                w = wqs_sb if swapped else wqn_sb
                b = bqs_sb if swapped else bq_sb
                ssl = slice(sc * QC, (sc + 1) * QC)
                ps = tx.tile([P, QC], f32, tag="tx", name="qps", bufs=2)
                for dt in range(N_DT):
                    nc.tensor.matmul(ps, lhsT=w[:, dt * P:(dt + 1) * P],
                                     rhs=qin_sb[:, sc, dt * QC:(dt + 1) * QC],
                                     start=(dt == 0), stop=(dt == N_DT - 1))
                if swapped:
                    nc.vector.tensor_scalar_add(
                        out=qh_sb[1][0:DK, ssl], in0=ps[0:DK, :],
                        scalar1=b[0:DK, :])
                    nc.vector.tensor_scalar_add(
                        out=qh_sb[0][DK:P, ssl], in0=ps[DK:P, :],
                        scalar1=b[DK:P, :])
                else:
                    nc.vector.tensor_scalar_add(
                        out=qh_sb[0][0:DK, ssl], in0=ps[0:DK, :],
                        scalar1=b[0:DK, :])
                    nc.vector.tensor_scalar_add(
                        out=qh_sb[1][DK:P, ssl], in0=ps[DK:P, :],
                        scalar1=b[DK:P, :])

            def vproj(vc):
                """Project V chunk vc, transpose per 128-key block into the
                per-head slot-ordered v tiles (masked, fp8)."""
                ps = tx.tile([P, QC], f32, tag="tx", name="vps", bufs=2)
                for dt in range(N_DT):
                    nc.tensor.matmul(ps, lhsT=wv_sb[:, dt * P:(dt + 1) * P],
                                     rhs=vin_sb[:, vc, dt * QC:(dt + 1) * QC],
                                     start=(dt == 0), stop=(dt == N_DT - 1))
                vt_sb = pin.tile([P, QC], bf16, tag="vt", bufs=2,
                                 name="vt_sb")
                nc.vector.tensor_scalar_add(out=vt_sb, in0=ps,
                                            scalar1=bv_sb)
                for st4 in range(QC // P):
                    g = vc * (QC // P) + st4
                    s0, s1 = _slot0(g, nkt), _slot1(g, nkt)
                    tp = tx.tile([P, P], bf16, tag="tx", name="tp", bufs=2)
                    nc.tensor.transpose(
                        tp, vt_sb[:, st4 * P:(st4 + 1) * P], ident_sb)
                    nc.vector.tensor_scalar_mul(
                        out=v0_sb[:, s0, 0:DK], in0=tp[:, 0:DK],
                        scalar1=mask0_sb[:, s0:s0 + 1])
                    nc.vector.tensor_scalar_mul(
                        out=v1_sb[:, s1, 0:DK], in0=tp[:, DK:P],
                        scalar1=mask1_sb[:, s1:s1 + 1])

            # initial projections in DMA-arrival order: B0 (gpsimd lands
            # first), Q chunk 0, A0, B-rest, A-rest.  V projections are
            # deferred into the attention loop (MM2s are software-
            # pipelined one step behind, so V has a full step of slack).
            kproj(n_hc)
            qproj(0, False)
            qproj(0, True)
            kproj(0)
            for i in range(1, n_hc):
                kproj(n_hc + i)
                kproj(i)

            # ================= attention =================
            # exp runs r in the current step while MM2/norm run one FULL
            # step behind (the PE queue is strict in-order, so an MM2
            # waiting on V or acc would block later MM1s and starve the
            # exp stream; one step of lag gives V/norm ~9us of slack).
            # Rounds 3 and 7 of each step compute exp on the DVE via a
            # Schraudolph bit-trick (bits = s*A + C viewed as bf16,
            # ~1.8% rms noise) to relieve the ACT engine.
            EXPA = float(128 * 0.125 * 1.4426950408889634)
            EXPC = float(16256 - 128 * 2 * 1.4426950408889634 - 7.41)
            DVE_ROUNDS = {3}
            u16 = mybir.dt.uint16
            with tc.tile_pool(name="aps", bufs=1, space="PSUM") as aps, \
                 tc.tile_pool(name="asb", bufs=1) as asb:
                v_sbs = (v0_sb, v1_sb)
                steps = [(h, sc) for h in (0, 1) for sc in range(N_SC)]
                NS = len(steps)

                def emit_mm1(h, sc, r):
                    """Row-tiled score pair for round r (slots 2r, 2r+1)."""
                    ssl = slice(sc * QC, (sc + 1) * QC)
                    s_ps = aps.tile([P, 2, QC], f32, tag="mm1", bufs=2,
                                    name="s_ps")
                    for j in (2 * r, 2 * r + 1):
                        p = j % 2
                        i = j // 2
                        psl = slice(DK * p, DK * (p + 1))
                        nc.tensor.matmul(
                            s_ps[:, p, :],
                            lhsT=kh_sb[h][psl, i * P:(i + 1) * P],
                            rhs=qh_sb[h][psl, ssl],
                            start=True, stop=True)
                    return s_ps

                def emit_outproj_tile(sc, st4):
                    st = sc * (QC // P) + st4
                    tsl = slice(st * P, (st + 1) * P)
                    po = tx.tile([P, D], f32, tag="tx", bufs=2,
                                 name="po")
                    nc.tensor.matmul(po, lhsT=a_sb[:, tsl],
                                     rhs=wos_sb, start=True, stop=True)
                    osb = asb.tile([P, D], bf16, tag="osb", bufs=4)
                    nc.any.tensor_copy(out=osb, in_=po)
                    eng = nc.gpsimd if st4 == 3 else nc.sync
                    eng.dma_start(out=OUT[tsl, :], in_=osb)

                p_store = {}
                s_ps_next = emit_mm1(0, 0, 0)
                acc = None
                pending = []
                for si in range(NS + 1):
                    cur = steps[si] if si < NS else None
                    prev = steps[si - 1] if si > 0 else None
                    for r in range(n_rounds):
                        if cur is not None:
                            h, sc = cur
                            s_ps = s_ps_next
                            p_sb = asb.tile([P, 2, QC], p_dt, tag="p",
                                            bufs=12)
                            p_store[(si, r)] = p_sb
                            if r in DVE_ROUNDS:
                                nc.vector.tensor_scalar(
                                    p_sb[:, :, :].bitcast(u16), s_ps,
                                    EXPA, EXPC,
                                    mybir.AluOpType.mult,
                                    mybir.AluOpType.add)
                            else:
                                nc.scalar.activation(
                                    p_sb, s_ps, EXP, bias=eshift_sb[:, :],
                                    scale=0.125)
                            if not (si == NS - 1 and r == n_rounds - 1):
                                nh, nsc = steps[si + 1] \
                                    if r == n_rounds - 1 else cur
                                nr = 0 if r == n_rounds - 1 else r + 1
                                s_ps_next = emit_mm1(nh, nsc, nr)

                        if prev is not None:
                            ph, psc = prev
                            # late V projections (DMA-arrival paced)
                            if si == 1:
                                if r == 0:
                                    vproj(0)
                                elif r == 3 and n_hc > 1:
                                    vproj(1)
                            if r == 0:
                                acc = aps.tile([65, QC], f32, tag="acc",
                                               bufs=2, name="acc")
                            pv = v_sbs[ph]
                            p_prev = p_store.pop((si - 1, r))
                            for j in (0, 1):
                                nc.tensor.matmul(
                                    acc[0:65, :],
                                    lhsT=pv[:, 2 * r + j, 0:65],
                                    rhs=p_prev[:, j, :],
                                    start=(r == 0 and j == 0),
                                    stop=(r == n_rounds - 1 and j == 1))
                            if r == n_rounds - 1:
                                # normalize prev step: attn = acc[0:64] /
                                # acc[64] via rank-1 f32r broadcast + fast
                                # reciprocal
                                pssl = slice(psc * QC, (psc + 1) * QC)
                                den = asb.tile([65, QC], f32r, tag="den",
                                               bufs=2)
                                nc.vector.tensor_copy(out=den[64:65, :],
                                                      in_=acc[64:65, :])
                                rb_ps = tx.tile([P, QC], f32, tag="tx",
                                                bufs=2, name="rb_ps")
                                nc.tensor.matmul(
                                    rb_ps[0:64, :],
                                    lhsT=ones64_sb[64:65, :],
                                    rhs=den[64:65, :],
                                    start=True, stop=True)
                                recb = asb.tile([64, QC], f32, tag="recb",
                                                bufs=2)
                                nc.vector.reciprocal_approx_fast(
                                    out=recb, in_=rb_ps[0:64, :])
                                nc.any.tensor_mul(
                                    out=a_sb[DK * ph:DK * (ph + 1), pssl],
                                    in0=acc[0:64, :], in1=recb)
                                if ph == 1:
                                    pending.append((psc, si))
                        if cur is not None:
                            h, sc = cur
                            # B-group V projections late in step 0
                            if si == 0:
                                if r == n_rounds - 2:
                                    vproj(n_hc)
                                elif r == n_rounds - 1 and n_hc > 1:
                                    vproj(n_hc + 1)
                            # mid-stream Q staging on the scalar queue
                            if si == 0 and r == 4 and N_SC > 2:
                                nc.scalar.dma_start(out=qin_sb[:, 2:3, :],
                                                    in_=QS[:, 2:3, :])
                            elif si == 0 and r == 6 and N_SC > 3:
                                nc.scalar.dma_start(out=qin_sb[:, 3:4, :],
                                                    in_=QS[:, 3:4, :])
                            elif si == 1 and r == 1 and N_SC > 4:
                                nc.scalar.dma_start(out=qin_sb[:, 4:5, :],
                                                    in_=QS[:, 4:5, :])
                            # next q chunk during h0: single projection +
                            # partition-dup via scalar-queue SBUF DMAs
                            if h == 0 and sc + 1 < N_SC and r == 2:
                                qproj(sc + 1, False)
                                nssl = slice((sc + 1) * QC, (sc + 2) * QC)
                                nc.scalar.dma_start(
                                    out=qh_sb[0][DK:P, nssl],
                                    in_=qh_sb[0][0:DK, nssl])
                                nc.scalar.dma_start(
                                    out=qh_sb[1][0:DK, nssl],
                                    in_=qh_sb[1][DK:P, nssl])
                    while pending and si - pending[0][1] >= 1:
                        for st4 in range(QC // P):
                            emit_outproj_tile(pending[0][0], st4)
                        pending.pop(0)
                for psc_o, _ in pending:
                    for st4 in range(QC // P):
                        emit_outproj_tile(psc_o, st4)
            pin.release()
            tx.release()

    nc.compile()
    return nc


def _get_compiled(nkt: int):
    if nkt not in _COMPILED:
        _COMPILED[nkt] = _build(nkt)
    return _COMPILED[nkt]


def kernel(Q, K, V, mask, W_q, b_q, W_k, b_k, W_v, b_v, W_o, b_o):
    from concourse import bass_utils

    bf16 = ml_dtypes.bfloat16
    mask = np.asarray(mask)
    if COMPACT:
        # keys with mask==0 contribute nothing: gather live keys, padded
        # to a multiple of 1024 (A/B halves of 512-key chunks)
        idxs = [np.flatnonzero(mask[b]) for b in range(B)]
        nkt = max(1, -(-max(len(ix) for ix in idxs) // P))
        nkt = min(-(-nkt // 8) * 8, S // P)
    else:
        idxs = None
        nkt = S // P
    nk = nkt * P

    Q = np.asarray(Q, np.float32)
    K = np.asarray(K, np.float32)
    V = np.asarray(V, np.float32)
    W_q = np.asarray(W_q, np.float32)
    W_k = np.asarray(W_k, np.float32)
    W_v = np.asarray(W_v, np.float32)
    W_o = np.asarray(W_o, np.float32)
    b_q = np.asarray(b_q, np.float32)
    b_k = np.asarray(b_k, np.float32)
    b_v = np.asarray(b_v, np.float32)
    b_o = np.asarray(b_o, np.float32)

    nc = _get_compiled(nkt)

    # slot permutation for the per-head mask tiles
    halft = nkt // 2
    slot0 = [0] * nkt
    for g in range(nkt):
        slot0[g] = 2 * g if g < halft else 2 * (g - halft) + 1
    slot1 = [0] * nkt
    for g in range(nkt):
        slot1[g] = 2 * g + 1 if g < halft else 2 * (g - halft)

    def stage_x(xt, nchunks):
        # [D, n] -> [P, chunk, N_DT*QC] with [p, c, dt*QC+col] =
        # xt[dt*P+p, c*QC+col]
        return np.ascontiguousarray(
            xt.reshape(N_DT, P, nchunks, QC).transpose(1, 2, 0, 3)
            .reshape(P, nchunks, N_DT * QC)).astype(bf16)

    def stage_w(w):
        # W slice [128out, 512in] -> [P, dt*P+c] = W.T[dt*P+p, c]
        return w.T.reshape(N_DT, P, P).transpose(1, 0, 2).reshape(P, D)

    n_kc = nk // QC
    in_maps = []
    for core in range(N_CORES):
        b = core // 4
        hp = core % 4
        rsl = slice(hp * P, (hp + 1) * P)   # rows of W_q/W_k/W_v, 2 heads
        h0, h1 = 2 * hp, 2 * hp + 1

        if COMPACT:
            ix = idxs[b]
            nkeep = len(ix)
            ixp = np.zeros(nk, np.int64)
            ixp[:nkeep] = ix[:nk]
            kt = K[b][ixp, :].T
            vt = V[b][ixp, :].T
            mvec = (np.arange(nk) < min(nkeep, nk)).astype(np.float32)
        else:
            kt = K[b].T
            vt = V[b].T
            mvec = mask[b].astype(np.float32)

        wq = W_q[rsl, :]                     # [128, 512], rows = head dims
        wk = W_k[rsl, :]
        swap = np.concatenate([np.arange(DK, P), np.arange(0, DK)])
        mt = mvec.reshape(nkt, P).T          # [128, nkt] global tile order
        # permute columns: MASKh[:, slot] = mask of global tile g
        m0 = np.zeros((P, nkt), np.float32)
        m1 = np.zeros((P, nkt), np.float32)
        for g in range(nkt):
            m0[:, slot0[g]] = mt[:, g]
            m1[:, slot1[g]] = mt[:, g]

        wos = np.zeros((P, D), np.float32)
        wos[0:DK, :] = W_o[:, h0 * DK:(h0 + 1) * DK].T
        wos[DK:P, :] = W_o[:, h1 * DK:(h1 + 1) * DK].T

        wall = np.stack([
            stage_w(wk), stage_w(wk[swap, :]),
            stage_w(wq), stage_w(wq[swap, :]),
            stage_w(W_v[rsl, :]), wos,
        ], axis=1)                            # [P, 6, D]

        small = np.zeros((P, 5 + 2 * nkt), np.float32)
        small[:, 0] = b_q[rsl]
        small[:, 1] = b_q[rsl][swap]
        small[:, 2] = b_k[rsl]
        small[:, 3] = b_k[rsl][swap]
        small[:, 4] = b_v[rsl]
        small[:, 5:5 + nkt] = m0
        small[:, 5 + nkt:] = m1

        in_maps.append({
            "QS": stage_x(Q[b].T, N_SC),
            "KS": stage_x(kt, n_kc),
            "VS": stage_x(vt, n_kc),
            "WALL": np.ascontiguousarray(wall).astype(bf16),
            "SMALL": small,
        })

    global _LAST_IN_MAPS, _LAST_RESULTS, _LAST_NKT
    _LAST_IN_MAPS = in_maps
    _LAST_NKT = nkt

    res = bass_utils.run_bass_kernel_spmd(
        nc, in_maps, core_ids=list(range(N_CORES)))

    _LAST_RESULTS = res.results

    out = np.zeros((B, S, D), np.float32)
    for core in range(N_CORES):
        out[core // 4] += np.asarray(res.results[core]["OUT"], np.float32)
    out += b_o[None, None, :]
    return out


# revision 62
# speedup vs baseline: 1.1558x; 1.0083x over previous
"""Multi-head attention (B=2, S=4096, D_MODEL=512, H=8) on 8 TRN2 NeuronCores.

Sharding (data + head/tensor parallel, per the problem's sharding hint):
each core owns (batch b = core//4, head-pair hp = core%4).
 - Q/K/V are batch-sharded (cores sharing b get the same activations,
   pre-transposed to [D, S] on the host so the PE can contract over D).
 - W_q/W_k/W_v are column-sharded per head pair (128 output dims/core).
 - W_o is row-sharded: each core emits a partial [S, D] output (bf16) and
   the host sums the four partials per batch + b_o (the "all-reduce"
   unshard of a row-sharded matmul).
 - Keys with mask==0 contribute nothing, so only live keys are gathered
   (padded to a multiple of 1024; pads carry mask 0 and cancel on-device).

Device pipeline per core (key performance structure):
  * Scores via ROW-TILED K=64 matmul pairs: the two heads' 64-dim q/k
    vectors live in partition halves.  Key-tiles are split into an A group
    (first nk/2 keys, projected with the natural weight layout -> head h
    dims at partitions 64h..64h+63) and a B group (second nk/2 keys,
    projected with a row-swapped weight copy -> head dims in the OTHER
    partition half).  Each score round issues one A-side MM (rows 0-63)
    and one B-side MM (rows 64-127); disjoint row-groups run concurrently
    on the PE, so MM1 streams ~256 score elements/cycle.
  * exp on ScalarE straight from 2 PSUM banks per call ([128,1024] bf16
    out, softmax scale 0.125 and a -2.0 shift folded into the activation);
    round 3 of each step computes exp on the DVE instead via a Schraudolph
    bit-trick (bits = s*A + C viewed as bf16, ~1.8% rms noise on 1/8 of
    the tiles) to relieve the ACT engine.
  * MM2 (p @ v, v column 64 = key mask yielding the softmax denominator
    as accumulator row 64) is software-pipelined one FULL step behind the
    exp stream: the PE queue is strict in-order, so an MM2 waiting on V or
    the accumulator would block later MM1s and starve exp.
  * Normalization: denominator broadcast across partitions with a rank-1
    f32r matmul + fast reciprocal; both heads' normalized rows land in one
    [128, S] stack (head0 rows 0-63, head1 rows 64-127).
  * Output projection: ONE matmul per 128-row s-tile (contraction over the
    full 128-dim stack against [W_o h0 | W_o h1]), bf16 partial out.
  * All bulk inputs are host-restaged into the exact SBUF layout so the
    few staging DMAs stream at line rate, ordered so the first exp fires
    ~17us in; the scalar DMA queue carries only 2 pre-attention DMAs
    (its ring-guard waits would block the exp stream in the ACT FIFO).
"""

import sys

for _p in ("/opt/trn_rl_repo", "/opt/pypackages"):
    if _p not in sys.path:
        sys.path.append(_p)

import numpy as np
import ml_dtypes

B = 2
S = 4096
D = 512
H = 8
DK = 64
N_CORES = 8

P = 128          # partitions
QC = 512         # q-chunk width
N_SC = S // QC   # 8 q-chunks
N_DT = D // P    # 4 D-tiles (contraction tiles for projections)

_COMPILED = {}
_LAST_IN_MAPS = None
_LAST_RESULTS = None
_LAST_NKT = None

COMPACT = True
# fp8 DoubleRow MM2 measured 3.9% output error (fp8's ~2% per-element
# noise on p and v passes straight through the softmax) vs the 2e-2
# gate -> bf16 per-tile MM2.
USE_DR = False


def _slot0(g, nkt):
    """v/mask slot for global key-tile g, head0 ordering."""
    half = nkt // 2
    return 2 * g if g < half else 2 * (g - half) + 1


def _slot1(g, nkt):
    half = nkt // 2
    return 2 * g + 1 if g < half else 2 * (g - half)


def _build(nkt: int):
    """Build + compile the per-core bass program. nkt = number of 128-wide
    key tiles (multiple of 8; 16 = 2048 live keys)."""
    import concourse.bass as bass  # noqa: F401
    from concourse.masks import make_identity
    import concourse.mybir as mybir
    import concourse.tile as tile
    from concourse import bacc

    f32 = mybir.dt.float32
    f32r = mybir.dt.float32r
    bf16 = mybir.dt.bfloat16
    fp8 = mybir.dt.float8e4
    EXP = mybir.ActivationFunctionType.Exp
    DR = mybir.MatmulPerfMode.DoubleRow

    p_dt = fp8 if USE_DR else bf16

    nk = nkt * P
    n_kc = nk // QC          # 512-key chunks (even; half A, half B)
    n_hc = n_kc // 2         # chunks per A/B group
    half_k = nk // 2
    n_rounds = nkt // 2      # k-rounds of 2 tiles

    nc = bacc.Bacc("TRN2", target_bir_lowering=False, debug=False,
                   enable_asserts=False)

    # All bulk inputs are pre-staged on the host into the exact SBUF
    # layout, so every staging DMA has multi-KB contiguous runs per
    # partition and streams at line rate (per-dt [128, cols] slices of a
    # [D, n] tensor only have 1-2KB runs -> descriptor-dominated, ~6us
    # per 256KB, which pushed the first exp out to ~47us).
    # [P, chunk, N_DT*QC]: chunk c, row p holds X.T[dt*128+p, c*512+col]
    QS = nc.dram_tensor("QS", [P, N_SC, N_DT * QC], bf16,
                        kind="ExternalInput").ap()
    KS = nc.dram_tensor("KS", [P, n_kc, N_DT * QC], bf16,
                        kind="ExternalInput").ap()
    VS = nc.dram_tensor("VS", [P, n_kc, N_DT * QC], bf16,
                        kind="ExternalInput").ap()
    # weights: slots [wkn, wks, wqn, wqs, wv] as [P, dt*128+c], slot 5 =
    # merged W_o stack [128, 512]
    WALL = nc.dram_tensor("WALL", [P, 6, D], bf16,
                          kind="ExternalInput").ap()
    # cols 0-4 = biases (bq, bqs, bk, bks, bv), then mask0, mask1
    SMALL = nc.dram_tensor("SMALL", [P, 5 + 2 * nkt], f32,
                           kind="ExternalInput").ap()
    OUT = nc.dram_tensor("OUT", [S, D], bf16, kind="ExternalOutput").ap()

    with tile.TileContext(nc) as tc:
        with tc.tile_pool(name="persist", bufs=1) as persist:
            # ---- persistent SBUF tensors ----
            # per-head q [128, S]: head h dims duplicated in BOTH partition
            # halves (A-side MMs read 0:64, B-side read 64:128)
            qh_sb = [persist.tile([P, S], bf16, name=f"qh{h}_sb")
                     for h in range(2)]
            # per-head k [128, nk/2]: partitions 0-63 = one key group's
            # head-h dims, 64-127 = the other group's (A/B per head)
            kh_sb = [persist.tile([P, half_k], bf16, name=f"kh{h}_sb")
                     for h in range(2)]
            # v tiles in slot order, padded inner dim 80 (DoubleRow needs
            # 16B-aligned pair stride); col 64 = key mask
            v0_sb = persist.tile([P, nkt, 80], p_dt)
            v1_sb = persist.tile([P, nkt, 80], p_dt)
            # normalized attention stack [h0 rows 0-63 | h1 rows 64-127]
            a_sb = persist.tile([P, S], bf16)
            # raw input staging (SBUF layout == DRAM layout, see above)
            kin_sb = persist.tile([P, n_kc, N_DT * QC], bf16)
            vin_sb = persist.tile([P, n_kc, N_DT * QC], bf16)
            qin_sb = persist.tile([P, N_SC, N_DT * QC], bf16)
            wall_sb = persist.tile([P, 6, D], bf16)
            small_sb = persist.tile([P, 5 + 2 * nkt], f32)
            ident_sb = persist.tile([P, P], bf16)
            ones64_sb = persist.tile([65, DK], f32r)
            ones64_f = persist.tile([65, DK], f32)
            dummy_sb = persist.tile([1, 16], f32)
            eshift_sb = persist.tile([P, 1], f32)   # exp bias (-2.0)
            nc.vector.memset(eshift_sb, -2.0)
            wkn_sb, wks_sb = wall_sb[:, 0, :], wall_sb[:, 1, :]
            wqn_sb, wqs_sb = wall_sb[:, 2, :], wall_sb[:, 3, :]
            wv_sb = wall_sb[:, 4, :]
            wos_sb = wall_sb[:, 5, :]
            bq_sb, bqs_sb = small_sb[:, 0:1], small_sb[:, 1:2]
            bk_sb, bks_sb = small_sb[:, 2:3], small_sb[:, 3:4]
            bv_sb = small_sb[:, 4:5]
            mask0_sb = small_sb[:, 5:5 + nkt]
            mask1_sb = small_sb[:, 5 + nkt:5 + 2 * nkt]

            # warm the ACT exp table during staging
            nc.vector.memset(dummy_sb, 0.0)
            nc.scalar.activation(dummy_sb, dummy_sb, EXP, bias=0.0,
                                 scale=1.0)
            make_identity(nc, ident_sb)
            nc.vector.memset(ones64_f, 1.0)
            nc.vector.tensor_copy(out=ones64_sb, in_=ones64_f)

            # ---------------- staging DMAs ----------------
            # Spread the critical path (weights -> K -> Q chunk 0) over
            # three queues, finest pieces first.  The scalar queue
            # carries only TWO pre-attention DMAs (its ring-guard waits
            # would otherwise block the exp stream in the ACT FIFO).
            # sync: W(kn,ks,qn,qs,v), K-A chunk 0, K-A rest, V-A0, V-A
            # rest, wos
            nc.sync.dma_start(out=wall_sb[:, 0:5, :], in_=WALL[:, 0:5, :])
            nc.sync.dma_start(out=kin_sb[:, 0:1, :], in_=KS[:, 0:1, :])
            if n_hc > 1:
                nc.sync.dma_start(out=kin_sb[:, 1:n_hc, :],
                                  in_=KS[:, 1:n_hc, :])
            nc.sync.dma_start(out=vin_sb[:, 0:1, :], in_=VS[:, 0:1, :])
            if n_hc > 1:
                nc.sync.dma_start(out=vin_sb[:, 1:n_hc, :],
                                  in_=VS[:, 1:n_hc, :])
            nc.sync.dma_start(out=wall_sb[:, 5, :], in_=WALL[:, 5, :])
            # gpsimd: smalls, K-B chunk 0, K-B rest, V-B0, V-B rest, Q 5-7
            nc.gpsimd.dma_start(out=small_sb, in_=SMALL)
            nc.gpsimd.dma_start(out=kin_sb[:, n_hc:n_hc + 1, :],
                                in_=KS[:, n_hc:n_hc + 1, :])
            if n_hc > 1:
                nc.gpsimd.dma_start(out=kin_sb[:, n_hc + 1:n_kc, :],
                                    in_=KS[:, n_hc + 1:n_kc, :])
            nc.gpsimd.dma_start(out=vin_sb[:, n_hc:n_hc + 1, :],
                                in_=VS[:, n_hc:n_hc + 1, :])
            if n_hc > 1:
                nc.gpsimd.dma_start(out=vin_sb[:, n_hc + 1:n_kc, :],
                                    in_=VS[:, n_hc + 1:n_kc, :])
            nc.gpsimd.dma_start(out=qin_sb[:, 5:N_SC, :],
                                in_=QS[:, 5:N_SC, :])
            # scalar: Q chunks 0-1 only
            nc.scalar.dma_start(out=qin_sb[:, 0:1, :], in_=QS[:, 0:1, :])
            nc.scalar.dma_start(out=qin_sb[:, 1:2, :], in_=QS[:, 1:2, :])

            # v mask columns (slot order, per head)
            nc.vector.tensor_copy(out=v0_sb[:, :, 64], in_=mask0_sb)
            nc.vector.tensor_copy(out=v1_sb[:, :, 64], in_=mask1_sb)

            # ================= projections =================
            pin = tc.alloc_tile_pool(name="pin", bufs=2)
            tx = tc.alloc_tile_pool(name="tx", bufs=2, space="PSUM")

            def kproj(c):
                """Project K chunk c (keys c*512..) into kh_0/kh_1."""
                swapped = c >= n_hc
                w = wks_sb if swapped else wkn_sb
                b = bks_sb if swapped else bk_sb
                osl = slice((c - n_hc) * QC, (c - n_hc + 1) * QC) \
                    if swapped else slice(c * QC, (c + 1) * QC)
                ps = tx.tile([P, QC], f32, tag="tx", name="kps", bufs=2)
                for dt in range(N_DT):
                    nc.tensor.matmul(ps, lhsT=w[:, dt * P:(dt + 1) * P],
                                     rhs=kin_sb[:, c, dt * QC:(dt + 1) * QC],
                                     start=(dt == 0), stop=(dt == N_DT - 1))
                if swapped:
                    # rows 0-63 = head1 dims, 64-127 = head0 dims
                    nc.vector.tensor_scalar_add(
                        out=kh_sb[1][0:DK, osl], in0=ps[0:DK, :],
                        scalar1=b[0:DK, :])
                    nc.vector.tensor_scalar_add(
                        out=kh_sb[0][DK:P, osl], in0=ps[DK:P, :],
                        scalar1=b[DK:P, :])
                else:
                    nc.vector.tensor_scalar_add(
                        out=kh_sb[0][0:DK, osl], in0=ps[0:DK, :],
                        scalar1=b[0:DK, :])
                    nc.vector.tensor_scalar_add(
                        out=kh_sb[1][DK:P, osl], in0=ps[DK:P, :],
                        scalar1=b[DK:P, :])

            def qproj(sc, swapped):
                """Project Q chunk sc with normal or swapped weights."""
                w = wqs_sb if swapped else wqn_sb
                b = bqs_sb if swapped else bq_sb
                ssl = slice(sc * QC, (sc + 1) * QC)
                ps = tx.tile([P, QC], f32, tag="tx", name="qps", bufs=2)
                for dt in range(N_DT):
                    nc.tensor.matmul(ps, lhsT=w[:, dt * P:(dt + 1) * P],
                                     rhs=qin_sb[:, sc, dt * QC:(dt + 1) * QC],
                                     start=(dt == 0), stop=(dt == N_DT - 1))
                if swapped:
                    nc.vector.tensor_scalar_add(
                        out=qh_sb[1][0:DK, ssl], in0=ps[0:DK, :],
                        scalar1=b[0:DK, :])
                    nc.vector.tensor_scalar_add(
                        out=qh_sb[0][DK:P, ssl], in0=ps[DK:P, :],
                        scalar1=b[DK:P, :])
                else:
                    nc.vector.tensor_scalar_add(
                        out=qh_sb[0][0:DK, ssl], in0=ps[0:DK, :],
                        scalar1=b[0:DK, :])
                    nc.vector.tensor_scalar_add(
                        out=qh_sb[1][DK:P, ssl], in0=ps[DK:P, :],
                        scalar1=b[DK:P, :])

            def vproj(vc):
                """Project V chunk vc, transpose per 128-key block into the
                per-head slot-ordered v tiles (masked, fp8)."""
                ps = tx.tile([P, QC], f32, tag="tx", name="vps", bufs=2)
                for dt in range(N_DT):
                    nc.tensor.matmul(ps, lhsT=wv_sb[:, dt * P:(dt + 1) * P],
                                     rhs=vin_sb[:, vc, dt * QC:(dt + 1) * QC],
                                     start=(dt == 0), stop=(dt == N_DT - 1))
                vt_sb = pin.tile([P, QC], bf16, tag="vt", bufs=2,
                                 name="vt_sb")
                nc.vector.tensor_scalar_add(out=vt_sb, in0=ps,
                                            scalar1=bv_sb)
                for st4 in range(QC // P):
                    g = vc * (QC // P) + st4
                    s0, s1 = _slot0(g, nkt), _slot1(g, nkt)
                    tp = tx.tile([P, P], bf16, tag="tx", name="tp", bufs=2)
                    nc.tensor.transpose(
                        tp, vt_sb[:, st4 * P:(st4 + 1) * P], ident_sb)
                    nc.vector.tensor_scalar_mul(
                        out=v0_sb[:, s0, 0:DK], in0=tp[:, 0:DK],
                        scalar1=mask0_sb[:, s0:s0 + 1])
                    nc.vector.tensor_scalar_mul(
                        out=v1_sb[:, s1, 0:DK], in0=tp[:, DK:P],
                        scalar1=mask1_sb[:, s1:s1 + 1])

            # initial projections in DMA-arrival order: B0 (gpsimd lands
            # first), Q chunk 0, A0, B-rest, A-rest.  V projections are
            # deferred into the attention loop (MM2s are software-
            # pipelined one step behind, so V has a full step of slack).
            kproj(n_hc)
            qproj(0, False)
            qproj(0, True)
            kproj(0)
            for i in range(1, n_hc):
                kproj(n_hc + i)
                kproj(i)

            # ================= attention =================
            # exp runs r in the current step while MM2/norm run one FULL
            # step behind (the PE queue is strict in-order, so an MM2
            # waiting on V or acc would block later MM1s and starve the
            # exp stream; one step of lag gives V/norm ~9us of slack).
            # Rounds 3 and 7 of each step compute exp on the DVE via a
            # Schraudolph bit-trick (bits = s*A + C viewed as bf16,
            # ~1.8% rms noise) to relieve the ACT engine.
            EXPA = float(128 * 0.125 * 1.4426950408889634)
            EXPC = float(16256 - 128 * 2 * 1.4426950408889634 - 7.41)
            DVE_ROUNDS = {3}
            u16 = mybir.dt.uint16
            with tc.tile_pool(name="aps", bufs=1, space="PSUM") as aps, \
                 tc.tile_pool(name="asb", bufs=1) as asb:
                v_sbs = (v0_sb, v1_sb)
                steps = [(h, sc) for h in (0, 1) for sc in range(N_SC)]
                NS = len(steps)

                def emit_mm1(h, sc, r):
                    """Row-tiled score pair for round r (slots 2r, 2r+1)."""
                    ssl = slice(sc * QC, (sc + 1) * QC)
                    s_ps = aps.tile([P, 2, QC], f32, tag="mm1", bufs=2,
                                    name="s_ps")
                    for j in (2 * r, 2 * r + 1):
                        p = j % 2
                        i = j // 2
                        psl = slice(DK * p, DK * (p + 1))
                        nc.tensor.matmul(
                            s_ps[:, p, :],
                            lhsT=kh_sb[h][psl, i * P:(i + 1) * P],
                            rhs=qh_sb[h][psl, ssl],
                            start=True, stop=True)
                    return s_ps

                def emit_outproj_tile(sc, st4):
                    st = sc * (QC // P) + st4
                    tsl = slice(st * P, (st + 1) * P)
                    po = tx.tile([P, D], f32, tag="tx", bufs=2,
                                 name="po")
                    nc.tensor.matmul(po, lhsT=a_sb[:, tsl],
                                     rhs=wos_sb, start=True, stop=True)
                    osb = asb.tile([P, D], bf16, tag="osb", bufs=4)
                    nc.any.tensor_copy(out=osb, in_=po)
                    eng = nc.gpsimd if st4 == 3 else nc.sync
                    eng.dma_start(out=OUT[tsl, :], in_=osb)

                p_store = {}
                s_ps_q = [emit_mm1(0, 0, 0)]
                acc = None
                pending = []
                for si in range(NS + 1):
                    cur = steps[si] if si < NS else None
                    prev = steps[si - 1] if si > 0 else None
                    for r in range(n_rounds):
                        if cur is not None:
                            h, sc = cur
                            s_ps = s_ps_q.pop(0)
                            p_sb = asb.tile([P, 2, QC], p_dt, tag="p",
                                            bufs=12)
                            p_store[(si, r)] = p_sb
                            if r in DVE_ROUNDS:
                                nc.vector.tensor_scalar(
                                    p_sb[:, :, :].bitcast(u16), s_ps,
                                    EXPA, EXPC,
                                    mybir.AluOpType.mult,
                                    mybir.AluOpType.add)
                            else:
                                nc.scalar.activation(
                                    p_sb, s_ps, EXP, bias=eshift_sb[:, :],
                                    scale=0.125)
                            # prefetch next rounds' scores; at
                            # r==n_rounds-2 also emit the NEXT step's
                            # round 0 so it sits in the PE FIFO ahead of
                            # the step-boundary MM2/norm/outproj burst
                            # (removes a ~0.7us ACT gap per step)
                            if r < n_rounds - 2:
                                s_ps_q.append(emit_mm1(h, sc, r + 1))
                            elif r == n_rounds - 2:
                                s_ps_q.append(
                                    emit_mm1(h, sc, n_rounds - 1))
                                if si + 1 < NS:
                                    nh, nsc = steps[si + 1]
                                    s_ps_q.append(emit_mm1(nh, nsc, 0))

                        if prev is not None:
                            ph, psc = prev
                            # late V projections (DMA-arrival paced)
                            if si == 1:
                                if r == 0:
                                    vproj(0)
                                elif r == 3 and n_hc > 1:
                                    vproj(1)
                            if r == 0:
                                acc = aps.tile([65, QC], f32, tag="acc",
                                               bufs=2, name="acc")
                            pv = v_sbs[ph]
                            p_prev = p_store.pop((si - 1, r))
                            for j in (0, 1):
                                nc.tensor.matmul(
                                    acc[0:65, :],
                                    lhsT=pv[:, 2 * r + j, 0:65],
                                    rhs=p_prev[:, j, :],
                                    start=(r == 0 and j == 0),
                                    stop=(r == n_rounds - 1 and j == 1))
                            if r == n_rounds - 1:
                                # normalize prev step: attn = acc[0:64] /
                                # acc[64] via rank-1 f32r broadcast + fast
                                # reciprocal
                                pssl = slice(psc * QC, (psc + 1) * QC)
                                den = asb.tile([65, QC], f32r, tag="den",
                                               bufs=2)
                                nc.vector.tensor_copy(out=den[64:65, :],
                                                      in_=acc[64:65, :])
                                rb_ps = tx.tile([P, QC], f32, tag="tx",
                                                bufs=2, name="rb_ps")
                                nc.tensor.matmul(
                                    rb_ps[0:64, :],
                                    lhsT=ones64_sb[64:65, :],
                                    rhs=den[64:65, :],
                                    start=True, stop=True)
                                recb = asb.tile([64, QC], f32, tag="recb",
                                                bufs=2)
                                nc.vector.reciprocal_approx_fast(
                                    out=recb, in_=rb_ps[0:64, :])
                                nc.any.tensor_mul(
                                    out=a_sb[DK * ph:DK * (ph + 1), pssl],
                                    in0=acc[0:64, :], in1=recb)
                                if ph == 1:
                                    pending.append((psc, si))
                        if cur is not None:
                            h, sc = cur
                            # B-group V projections late in step 0
                            if si == 0:
                                if r == n_rounds - 2:
                                    vproj(n_hc)
                                elif r == n_rounds - 1 and n_hc > 1:
                                    vproj(n_hc + 1)
                            # mid-stream Q staging on the scalar queue
                            if si == 0 and r == 4 and N_SC > 2:
                                nc.scalar.dma_start(out=qin_sb[:, 2:3, :],
                                                    in_=QS[:, 2:3, :])
                            elif si == 0 and r == 6 and N_SC > 3:
                                nc.scalar.dma_start(out=qin_sb[:, 3:4, :],
                                                    in_=QS[:, 3:4, :])
                            elif si == 1 and r == 1 and N_SC > 4:
                                nc.scalar.dma_start(out=qin_sb[:, 4:5, :],
                                                    in_=QS[:, 4:5, :])
                            # next q chunk during h0: single projection +
                            # partition-dup via scalar-queue SBUF DMAs
                            if h == 0 and sc + 1 < N_SC and r == 2:
                                qproj(sc + 1, False)
                                nssl = slice((sc + 1) * QC, (sc + 2) * QC)
                                nc.scalar.dma_start(
                                    out=qh_sb[0][DK:P, nssl],
                                    in_=qh_sb[0][0:DK, nssl])
                                nc.scalar.dma_start(
                                    out=qh_sb[1][0:DK, nssl],
                                    in_=qh_sb[1][DK:P, nssl])
                    while pending and si - pending[0][1] >= 1:
                        for st4 in range(QC // P):
                            emit_outproj_tile(pending[0][0], st4)
                        pending.pop(0)
                for psc_o, _ in pending:
                    for st4 in range(QC // P):
                        emit_outproj_tile(psc_o, st4)
            pin.release()
            tx.release()

    nc.compile()
    return nc


def _get_compiled(nkt: int):
    if nkt not in _COMPILED:
        _COMPILED[nkt] = _build(nkt)
    return _COMPILED[nkt]


def kernel(Q, K, V, mask, W_q, b_q, W_k, b_k, W_v, b_v, W_o, b_o):
    from concourse import bass_utils

    bf16 = ml_dtypes.bfloat16
    mask = np.asarray(mask)
    if COMPACT:
        # keys with mask==0 contribute nothing: gather live keys, padded
        # to a multiple of 1024 (A/B halves of 512-key chunks)
        idxs = [np.flatnonzero(mask[b]) for b in range(B)]
        nkt = max(1, -(-max(len(ix) for ix in idxs) // P))
        nkt = min(-(-nkt // 8) * 8, S // P)
    else:
        idxs = None
        nkt = S // P
    nk = nkt * P

    Q = np.asarray(Q, np.float32)
    K = np.asarray(K, np.float32)
    V = np.asarray(V, np.float32)
    W_q = np.asarray(W_q, np.float32)
    W_k = np.asarray(W_k, np.float32)
    W_v = np.asarray(W_v, np.float32)
    W_o = np.asarray(W_o, np.float32)
    b_q = np.asarray(b_q, np.float32)
    b_k = np.asarray(b_k, np.float32)
    b_v = np.asarray(b_v, np.float32)
    b_o = np.asarray(b_o, np.float32)

    nc = _get_compiled(nkt)

    # slot permutation for the per-head mask tiles
    halft = nkt // 2
    slot0 = [0] * nkt
    for g in range(nkt):
        slot0[g] = 2 * g if g < halft else 2 * (g - halft) + 1
    slot1 = [0] * nkt
    for g in range(nkt):
        slot1[g] = 2 * g + 1 if g < halft else 2 * (g - halft)

    def stage_x(xt, nchunks):
        # [D, n] -> [P, chunk, N_DT*QC] with [p, c, dt*QC+col] =
        # xt[dt*P+p, c*QC+col]
        return np.ascontiguousarray(
            xt.reshape(N_DT, P, nchunks, QC).transpose(1, 2, 0, 3)
            .reshape(P, nchunks, N_DT * QC)).astype(bf16)

    def stage_w(w):
        # W slice [128out, 512in] -> [P, dt*P+c] = W.T[dt*P+p, c]
        return w.T.reshape(N_DT, P, P).transpose(1, 0, 2).reshape(P, D)

    n_kc = nk // QC
    in_maps = []
    for core in range(N_CORES):
        b = core // 4
        hp = core % 4
        rsl = slice(hp * P, (hp + 1) * P)   # rows of W_q/W_k/W_v, 2 heads
        h0, h1 = 2 * hp, 2 * hp + 1

        if COMPACT:
            ix = idxs[b]
            nkeep = len(ix)
            ixp = np.zeros(nk, np.int64)
            ixp[:nkeep] = ix[:nk]
            kt = K[b][ixp, :].T
            vt = V[b][ixp, :].T
            mvec = (np.arange(nk) < min(nkeep, nk)).astype(np.float32)
        else:
            kt = K[b].T
            vt = V[b].T
            mvec = mask[b].astype(np.float32)

        wq = W_q[rsl, :]                     # [128, 512], rows = head dims
        wk = W_k[rsl, :]
        swap = np.concatenate([np.arange(DK, P), np.arange(0, DK)])
        mt = mvec.reshape(nkt, P).T          # [128, nkt] global tile order
        # permute columns: MASKh[:, slot] = mask of global tile g
        m0 = np.zeros((P, nkt), np.float32)
        m1 = np.zeros((P, nkt), np.float32)
        for g in range(nkt):
            m0[:, slot0[g]] = mt[:, g]
            m1[:, slot1[g]] = mt[:, g]

        wos = np.zeros((P, D), np.float32)
        wos[0:DK, :] = W_o[:, h0 * DK:(h0 + 1) * DK].T
        wos[DK:P, :] = W_o[:, h1 * DK:(h1 + 1) * DK].T

        wall = np.stack([
            stage_w(wk), stage_w(wk[swap, :]),
            stage_w(wq), stage_w(wq[swap, :]),
            stage_w(W_v[rsl, :]), wos,
        ], axis=1)                            # [P, 6, D]

        small = np.zeros((P, 5 + 2 * nkt), np.float32)
        small[:, 0] = b_q[rsl]
        small[:, 1] = b_q[rsl][swap]
        small[:, 2] = b_k[rsl]
        small[:, 3] = b_k[rsl][swap]
        small[:, 4] = b_v[rsl]
        small[:, 5:5 + nkt] = m0
        small[:, 5 + nkt:] = m1

        in_maps.append({
            "QS": stage_x(Q[b].T, N_SC),
            "KS": stage_x(kt, n_kc),
            "VS": stage_x(vt, n_kc),
            "WALL": np.ascontiguousarray(wall).astype(bf16),
            "SMALL": small,
        })

    global _LAST_IN_MAPS, _LAST_RESULTS, _LAST_NKT
    _LAST_IN_MAPS = in_maps
    _LAST_NKT = nkt

    res = bass_utils.run_bass_kernel_spmd(
        nc, in_maps, core_ids=list(range(N_CORES)))

    _LAST_RESULTS = res.results

    out = np.zeros((B, S, D), np.float32)
    for core in range(N_CORES):
        out[core // 4] += np.asarray(res.results[core]["OUT"], np.float32)
    out += b_o[None, None, :]
    return out
